# revision 1
# baseline (speedup 1.0000x reference)
"""Trainium2 Bass kernel for nn_ClaimEncoder (dense_mlp).

Math (per row):
  feats = [sin/cos point-encoders (2x256), leaky number-encoders (3x128)]  -> [896]
  h   = leaky_relu(feats @ W1 + b1)   -> [512]
  out = leaky_relu(h @ W2 + b2)       -> [512]

Strategy: pure data parallel over 8 NeuronCores (16384 rows each).

Device-side design (per core, batch tiles of NB=512 columns):
  * The encoder never touches the PE. For each 128-feature chunk a DMA
    partition-broadcast replicates the batch-value row; sin/cos chunks:
    the idle Pool engine computes z' = (w[p]*v + b[p])/2pi (cos =
    sin(z+pi/2) via the bias), the DVE range-reduces with the fp32
    magic-constant rounding trick (k = round(z'), y = k - z'; ACT Sin
    only accepts [-pi, pi]) and ACT computes sin(-2pi*y).  Number
    chunks: one ACT op, prelu(w[p]*v + b[p]) via per-partition
    scale/bias.
  * featsT comes out feature-major [feat, batch] - exactly the K-layout
    the L1 matmul needs for both operands.  L1: hT = W1_chunk.T @ featsT
    (bias b1 + leaky fused into the ACT eviction, b1 is per-partition).
  * L2 uses hT as the *stationary* operand (lhsT = hT chunk, rhs = W2
    chunk) which lands the output batch-major in PSUM -> contiguous DMA
    to DRAM.  b2 varies along the free dim so DVE adds broadcast b2
    during eviction, ACT applies the leaky relu.
  * All matmuls use float32r (1 cycle/row at N=512 vs 4 for plain fp32).
  * The emission loop runs one tile skewed (enc(t+1) before mlp(t)) so
    the encoder's Pool->DVE->ACT chain hides under tile t's MLP
    matmuls.  The PE runs only the 44 L1/L2 matmuls per tile; all five
    engines carry load (PE ~300us, ACT ~295us, DVE ~253us per core).
"""

import numpy as np

import concourse.bass as bass
import concourse.tile as tile
import concourse.mybir as mybir
from concourse import bacc
from concourse.bass_utils import run_bass_kernel_spmd

# Problem shapes (hardcoded; kernel.py must be self-contained).
B = 131072
N_CORES = 8
BC = B // N_CORES          # 16384 rows per core
PED = 256
NED = 128
CED = 512
Q = PED // 4               # 64
FEAT = 2 * PED + 3 * NED   # 896
NB = 512                   # batch columns per matmul tile
N_TILES = BC // NB         # 32
KC = FEAT // 128           # 7 feature chunks
MC = CED // 128            # 4 output chunks

TWO_PI = 2.0 * np.pi
# fp32 round-to-nearest-integer magic constant: adding it forces the
# mantissa to integer granularity (valid for |x| << 2^22).
MAGIC = 1.5 * 2.0 ** 23

F32 = mybir.dt.float32
F32R = mybir.dt.float32r


def _build_bass():
    nc = bacc.Bacc(
        "TRN2",
        target_bir_lowering=False,
        debug=False,
        enable_asserts=False,
        num_devices=N_CORES,
    )

    a8 = nc.dram_tensor("a8", [8, BC], F32R, kind="ExternalInput").ap()
    w1 = nc.dram_tensor("w1", [FEAT, CED], F32R, kind="ExternalInput").ap()
    w2 = nc.dram_tensor("w2", [CED, CED], F32R, kind="ExternalInput").ap()
    b1 = nc.dram_tensor("b1", [CED], F32, kind="ExternalInput").ap()
    b2 = nc.dram_tensor("b2", [CED], F32, kind="ExternalInput").ap()
    # number-encoder per-feature (w, b) pairs: cols [t_w,t_b,ws_w,ws_b,wd_w,wd_b]
    nwb = nc.dram_tensor("nwb", [128, 6], F32, kind="ExternalInput").ap()
    # point-encoder per-feature (w, b)/2pi pairs, cols [w_c, b_c] for c=0..3
    pwb = nc.dram_tensor("pwb", [128, 8], F32, kind="ExternalInput").ap()
    out = nc.dram_tensor("out", [BC, CED], F32, kind="ExternalOutput").ap()

    with tile.TileContext(nc) as tc:
        with (
            tc.tile_pool(name="consts", bufs=1) as consts,
            tc.tile_pool(name="featsp", bufs=2) as feats_pool,
            tc.tile_pool(name="hp", bufs=2) as h_pool,
            tc.tile_pool(name="rrp", bufs=4) as rr_pool,
            tc.tile_pool(name="l2tmp", bufs=4) as l2tmp_pool,
            tc.tile_pool(name="outp", bufs=6) as out_pool,
            tc.tile_pool(name="l1_ps", bufs=4, space="PSUM") as l1_psum,
            tc.tile_pool(name="l2_ps", bufs=4, space="PSUM") as l2_psum,
        ):
            w1_sb = consts.tile([128, KC * CED], F32R)
            for c in range(KC):
                nc.sync.dma_start(
                    out=w1_sb[:, c * CED:(c + 1) * CED],
                    in_=w1[c * 128:(c + 1) * 128, :],
                )
            w2_sb = consts.tile([128, MC * CED], F32R)
            for k in range(MC):
                nc.sync.dma_start(
                    out=w2_sb[:, k * CED:(k + 1) * CED],
                    in_=w2[k * 128:(k + 1) * 128, :],
                )
            # b1 per-chunk columns: b1_sb[p, m] = b1[m*128 + p]
            b1_sb = consts.tile([128, MC], F32)
            nc.sync.dma_start(out=b1_sb[:], in_=b1.rearrange("(m q) -> q m", q=128))
            # b2 broadcast across partitions: b2b[p, f] = b2[f]
            b2b_sb = consts.tile([128, CED], F32)
            b2_bcast = bass.AP(
                tensor=b2.tensor, offset=b2.offset, ap=[[0, 128]] + list(b2.ap)
            )
            nc.gpsimd.dma_start(out=b2b_sb[:], in_=b2_bcast)
            nwb_sb = consts.tile([128, 6], F32)
            nc.sync.dma_start(out=nwb_sb[:], in_=nwb[:, :])
            pwb_sb = consts.tile([128, 8], F32)
            nc.sync.dma_start(out=pwb_sb[:], in_=pwb[:, :])

            feats_tiles = {}

            def bcast_row(t, r, tag):
                """DMA-replicate a8[r, tile t] across 128 partitions."""
                vb = rr_pool.tile([128, NB], F32, name=f"vb{tag}_{t}_{r}", tag=f"vb{tag}")
                src_ap = bass.AP(
                    tensor=a8.tensor, offset=r * BC + t * NB,
                    ap=[[0, 128], [1, NB]],
                ).bitcast(F32)
                nc.sync.dma_start(out=vb[:], in_=src_ap)
                return vb

            def emit_enc(t):
                """Encoder for tile t: no PE at all (broadcast + Pool/DVE/ACT)."""
                feats = feats_pool.tile([128, KC * NB], F32R,
                                        name=f"feats_{t}", tag="feats")
                feats_tiles[t] = feats
                for c in range(4):
                    dst = feats[:, c * NB:(c + 1) * NB]
                    vb = bcast_row(t, c, "s")
                    # Pool computes z' = (w[p]*v + b[p])  (pre-scaled by 1/2pi);
                    # DVE range-reduces: k = round(z'), y = k - z'; ACT does
                    # sin(-2pi*y) = sin(z).
                    zp = rr_pool.tile([128, NB], F32, name=f"zp_{t}_{c}", tag="zp")
                    nc.gpsimd.tensor_scalar(
                        zp[:], vb[:], pwb_sb[:, 2 * c:2 * c + 1],
                        pwb_sb[:, 2 * c + 1:2 * c + 2],
                        op0=mybir.AluOpType.mult, op1=mybir.AluOpType.add,
                    )
                    rr = rr_pool.tile([128, NB], F32, name=f"rr_{t}_{c}", tag="rr")
                    nc.vector.tensor_scalar_add(rr[:], zp[:], MAGIC)
                    rr2 = rr_pool.tile([128, NB], F32, name=f"rr2_{t}_{c}", tag="rr2")
                    nc.vector.scalar_tensor_tensor(
                        rr2[:], rr[:], MAGIC, zp[:],
                        op0=mybir.AluOpType.subtract,
                        op1=mybir.AluOpType.subtract,
                    )
                    nc.scalar.activation(
                        dst, rr2[:], mybir.ActivationFunctionType.Sin,
                        scale=-TWO_PI,
                    )
                # Number-encoder chunks: no PE needed. DMA replicates the
                # value row across 128 partitions; ACT applies
                # prelu(w[p]*v + b[p]) with per-partition scale/bias.
                for i in range(3):
                    dst = feats[:, (4 + i) * NB:(5 + i) * NB]
                    vb = bcast_row(t, 4 + i, "n")
                    nc.scalar.activation(
                        dst, vb[:], mybir.ActivationFunctionType.Prelu,
                        scale=nwb_sb[:, 2 * i:2 * i + 1],
                        bias=nwb_sb[:, 2 * i + 1:2 * i + 2],
                        alpha=0.01,
                    )

            def emit_mlp(t):
                """L1 + L2 + store for tile t (consumes feats_tiles[t])."""
                bt = t * NB
                feats = feats_tiles.pop(t)
                h = h_pool.tile([128, MC * NB], F32R, name=f"h_{t}", tag="h")
                for m in range(MC):
                    l1p = l1_psum.tile([128, NB], F32, name=f"l1p_{t}_{m}", tag="l1p")
                    for c in range(KC):
                        nc.tensor.matmul(
                            l1p[:],
                            w1_sb[:, c * CED + m * 128: c * CED + (m + 1) * 128],
                            feats[:, c * NB:(c + 1) * NB],
                            start=(c == 0),
                            stop=(c == KC - 1),
                        )
                    nc.scalar.activation(
                        h[:, m * NB:(m + 1) * NB], l1p[:],
                        mybir.ActivationFunctionType.Prelu,
                        bias=b1_sb[:, m:m + 1], alpha=0.01,
                    )

                for j in range(MC):
                    l2p = l2_psum.tile([128, NB], F32, name=f"l2p_{t}_{j}", tag="l2p")
                    for k in range(MC):
                        nc.tensor.matmul(
                            l2p[:],
                            h[:, k * NB + j * 128: k * NB + (j + 1) * 128],
                            w2_sb[:, k * CED:(k + 1) * CED],
                            start=(k == 0),
                            stop=(k == MC - 1),
                        )
                    l2t = l2tmp_pool.tile([128, NB], F32, name=f"l2t_{t}_{j}", tag="l2t")
                    nc.vector.tensor_tensor(
                        l2t[:], l2p[:], b2b_sb[:], op=mybir.AluOpType.add
                    )
                    osb = out_pool.tile([128, NB], F32, name=f"osb_{t}_{j}", tag="osb")
                    nc.scalar.activation(
                        osb[:], l2t[:], mybir.ActivationFunctionType.Prelu, alpha=0.01
                    )
                    nc.sync.dma_start(
                        out=out[bt + j * 128: bt + (j + 1) * 128, :], in_=osb[:]
                    )

            # Software pipeline with one-tile skew: PE runs enc(t+1) before
            # L1/L2(t), so the DVE->ACT eviction latency of tile t+1's
            # features hides under tile t's MLP matmuls.
            emit_enc(0)
            for t in range(1, N_TILES):
                emit_enc(t)
                emit_mlp(t - 1)
            emit_mlp(N_TILES - 1)

    nc.compile()
    return nc


def _host_pack(inputs):
    """Build A8 [8, B] and the encoder matrix P [8, FEAT] (bias folded)."""
    f32 = lambda k: np.ascontiguousarray(np.asarray(inputs[k], dtype=np.float32))
    src = f32("src_xy")
    dst = f32("dst_xy")

    a8 = np.empty((8, B), np.float32)
    a8[0] = src[:, 0]
    a8[1] = src[:, 1]
    a8[2] = dst[:, 0]
    a8[3] = dst[:, 1]
    a8[4] = f32("time_s")
    a8[5] = f32("wait_src")
    a8[6] = f32("wait_dst")
    a8[7] = 1.0

    # point-encoder (w, b)/2pi per feature: [128, 8], col pair per chunk.
    # chunk c partition p: p<64 -> sin block, p>=64 -> cos block (+pi/2).
    pwb = np.empty((128, 8), np.float32)
    for c, (pfx, ax) in enumerate((("src", "x"), ("src", "y"),
                                   ("dst", "x"), ("dst", "y"))):
        pwb[:64, 2 * c] = f32(f"{pfx}_ws{ax}") / TWO_PI
        pwb[:64, 2 * c + 1] = f32(f"{pfx}_bs{ax}") / TWO_PI
        pwb[64:, 2 * c] = f32(f"{pfx}_wc{ax}") / TWO_PI
        pwb[64:, 2 * c + 1] = (f32(f"{pfx}_bc{ax}") + np.pi / 2) / TWO_PI
    # number-encoder (w, b) per feature: [128, 6] cols t_w,t_b,ws_w,ws_b,wd_w,wd_b
    nwb = np.empty((128, 6), np.float32)
    for i, pfx in enumerate(("t", "ws", "wd")):
        nwb[:, 2 * i] = f32(f"{pfx}_w")
        nwb[:, 2 * i + 1] = f32(f"{pfx}_b")

    w1 = f32("W1")
    b1 = f32("b1")
    w2 = f32("W2")
    b2 = f32("b2")
    return a8, pwb, nwb, w1, b1, w2, b2


_NC_CACHE = []


def kernel(**inputs) -> np.ndarray:
    a8, pwb, nwb, w1, b1, w2, b2 = _host_pack(inputs)

    if not _NC_CACHE:
        _NC_CACHE.append(_build_bass())
    nc = _NC_CACHE[0]

    in_maps = []
    for i in range(N_CORES):
        in_maps.append({
            "a8": np.ascontiguousarray(a8[:, i * BC:(i + 1) * BC]),
            "pwb": pwb,
            "w1": w1,
            "w2": w2,
            "b1": b1,
            "b2": b2,
            "nwb": nwb,
        })

    res = run_bass_kernel_spmd(nc, in_maps, core_ids=list(range(N_CORES)))
    return np.concatenate([r["out"] for r in res.results], axis=0)

